# revision 10
# baseline (speedup 1.0000x reference)
"""Complex-valued fully-connected layer on 8 TRN2 NeuronCores.

Math (per reference):
    out_re = x_re @ w_re^T - x_im @ w_im^T
    out_im = x_re @ w_im^T + x_im @ w_re^T        -> stack([out_re, out_im])
with x_*: [8192, 2048] f32, w_*: [2048, 2048] f32.

Strategy (v2):
  - Shard 8 cores = 2 batch-halves (4096 rows) x 4 out-feature quarters (512).
    Outputs are disjoint -> no collectives.
  - Karatsuba (3 real GEMMs instead of 4):
        t_rr = x_re @ w_re^T ; t_ii = x_im @ w_im^T ; t_ss = (x_re+x_im)@(w_re+w_im)^T
        out_re = t_rr - t_ii ; out_im = t_ss - t_rr - t_ii
  - All matmul operands cast to bf16 on host (full-rate PE, ~3e-3 rel err,
    half the DMA bytes). PSUM accumulates fp32; outputs stored fp32.
  - x_s = x_re + x_im and w_s = w_re + w_im are precomputed on HOST, so no
    on-device elementwise work sits upstream of the PE: the only DVE work is
    the per-tile PSUM combine, which trails the matmuls.
  - Host pre-arranges x as [tile, partition, k, col] so every x-tile DMA is
    one 0.5 MB contiguous block with 4 KB-per-partition descriptors (>=512B
    keeps the DMA model at full rate; a plain [k,b] layout would give 256B
    descriptors at half rate).
  - PE p-state: the Tensor engine clocks down whenever its pipeline drains
    and takes ~3us to re-ramp. The schedule keeps every PE semaphore wait
    pre-satisfied in steady state, and a burst of warm-up matmuls on a
    zeroed SBUF scratch ramps the PE while the first weight chunks stream.
  - Engine roles: SP = x-tile loads, ACT = weight loads + output stores,
    DVE = PSUM combines (+ warmup memset). Pool/GPSIMD idle.
"""

import numpy as np
import ml_dtypes

import concourse.bass as bass
from concourse import mybir
from concourse.bass_utils import run_bass_kernel_spmd

BATCH, IN_F, OUT_F = 8192, 2048, 2048
N_CORES = 8
B_SHARDS, O_SHARDS = 2, 4
B_SH = BATCH // B_SHARDS          # 4096 batch rows per core
O_SH = OUT_F // O_SHARDS          # 512 out features per core
KT = IN_F // 128                  # 16 contraction tiles
BT = B_SH // 128                  # 32 batch tiles per core
XD = 6                            # x-tile SBUF buffer depth
OD = 4                            # output staging depth
WCH = 4                           # k-tiles per weight-load chunk (4 chunks/mat)
NWC = KT // WCH                   # chunks per weight matrix
N_WARM = 19                       # PE warm-up matmuls

F32 = mybir.dt.float32
BF16 = mybir.dt.bfloat16
BF16_NP = ml_dtypes.bfloat16


def build_nc() -> bass.Bass:
    nc = bass.Bass("TRN2", target_bir_lowering=False, debug=False)

    # x pre-arranged on host: [tile, partition, k, col] so one tile is a
    # single contiguous 0.5MB DMA with 4KB-per-partition descriptors.
    x_re_d = nc.dram_tensor("x_re_d", [BT, 128, KT, 128], BF16, kind="ExternalInput")
    x_im_d = nc.dram_tensor("x_im_d", [BT, 128, KT, 128], BF16, kind="ExternalInput")
    x_s_d = nc.dram_tensor("x_s_d", [BT, 128, KT, 128], BF16, kind="ExternalInput")
    # weights: [in, out] slices; contiguous 1KB rows
    wt_re = nc.dram_tensor("wt_re", [IN_F, O_SH], BF16, kind="ExternalInput")
    wt_im = nc.dram_tensor("wt_im", [IN_F, O_SH], BF16, kind="ExternalInput")
    wt_s = nc.dram_tensor("wt_s", [IN_F, O_SH], BF16, kind="ExternalInput")
    out_d = nc.dram_tensor("out", [2, B_SH, O_SH], F32, kind="ExternalOutput")

    w_re_sb = nc.alloc_sbuf_tensor("w_re_sb", [128, KT, O_SH], BF16)
    w_im_sb = nc.alloc_sbuf_tensor("w_im_sb", [128, KT, O_SH], BF16)
    w_s_sb = nc.alloc_sbuf_tensor("w_s_sb", [128, KT, O_SH], BF16)
    x_re_sb = nc.alloc_sbuf_tensor("x_re_sb", [128, XD, KT, 128], BF16)
    x_im_sb = nc.alloc_sbuf_tensor("x_im_sb", [128, XD, KT, 128], BF16)
    x_s_sb = nc.alloc_sbuf_tensor("x_s_sb", [128, XD, KT, 128], BF16)
    o_re_sb = nc.alloc_sbuf_tensor("o_re_sb", [128, OD, O_SH], F32)
    o_im_sb = nc.alloc_sbuf_tensor("o_im_sb", [128, OD, O_SH], F32)
    r_sb = nc.alloc_sbuf_tensor("r_sb", [128, 4, O_SH], F32)  # t_rr staging
    s2_sb = nc.alloc_sbuf_tensor("s2_sb", [128, 4, O_SH], F32)  # rr+ii staging
    warm_sb = nc.alloc_sbuf_tensor("warm_sb", [128, O_SH], BF16)

    p_rr4 = [nc.alloc_psum_tensor(f"p_rr{b}", [128, O_SH], F32) for b in range(4)]
    p_ii2 = [nc.alloc_psum_tensor(f"p_ii{b}", [128, O_SH], F32) for b in range(2)]
    p_ss2 = [nc.alloc_psum_tensor(f"p_ss{b}", [128, O_SH], F32) for b in range(2)]
    # warm-up matmuls write an ss bank; its accumulation group is closed
    # (stop=True) before ss(1) restarts the bank with start=True.
    p_warm = p_ss2[1]

    wt_re_r = wt_re.ap().rearrange("(k p) o -> p k o", p=128)
    wt_im_r = wt_im.ap().rearrange("(k p) o -> p k o", p=128)
    wt_s_r = wt_s.ap().rearrange("(k p) o -> p k o", p=128)

    # --- global PE group sequence -----------------------------------------
    # Tiles 0-3 run their rr group first (needs only w_re), so PE has work
    # while w_im / w_s stream through the serial DMA-engine pool:
    #   idx 0..3         rr(0..3)
    #   idx 4..7         ii(0..3)
    #   idx 8..11        ss(0..3)
    #   idx 12+3(t-4)+g  rr/ii/ss(t)           for t >= 4
    def g_rr(t):
        return t + 1 if t <= 3 else 13 + 3 * (t - 4)

    def g_ii(t):
        return 5 + t if t <= 3 else 14 + 3 * (t - 4)

    def g_ss(t):
        return 9 + t if t <= 3 else 15 + 3 * (t - 4)

    # PSUM bank assignment: rr 4-deep, ii / ss 2-deep each (+1 warm-up bank
    # shared with ss odd tiles — warm-up's accumulation group is closed
    # before ss(1) restarts the bank with start=True).
    def p_rr_of(t):
        return p_rr4[t % 4]

    def p_ii_of(t):
        return p_ii2[t % 2]

    def p_ss_of(t):
        return p_ss2[t % 2]

    # ACT-ring startup order == exact PE consumption order. Entry g of the
    # startup x list lands as dma_x0 = 16*(g+1).
    STARTUP_X = (  # (sem slot order) tensor kind, tile
        ("re", 0), ("re", 1), ("re", 2), ("re", 3),
        ("im", 0), ("im", 1), ("im", 2), ("im", 3),
        ("s", 0), ("s", 1), ("s", 2), ("s", 3),
    )
    X0_SLOT = {kt: i + 1 for i, kt in enumerate(STARTUP_X)}

    with (
        nc.Block() as block,
        nc.semaphore("dma_x") as dma_x,      # SP ring: +16 per x tile DMA (3/tile, t>=4)
        nc.semaphore("dma_x0") as dma_x0,    # ACT ring: +16 per startup x tile (12)
        nc.semaphore("dma_w") as dma_w,      # ACT ring: +16 per weight chunk (12)
        nc.semaphore("mm_done") as mm_done,  # PE: +1 per matmul group (global seq)
        nc.semaphore("cmb_ii") as cmb_ii,    # DVE: +1 when p_ii(t) fully read
        nc.semaphore("cmb_done") as cmb_done,  # DVE: +1 per tile combined
        nc.semaphore("dma_out") as dma_out,  # ACT ring: +16 per output store
        nc.semaphore("warm_done") as warm_done,  # DVE: warm_sb zeroed
    ):

        @block.sync
        def _(sp):
            # steady-state x loads, tiles 4+. Pace the first issue behind the
            # startup stream so these DMAs don't jump the serial pool queue.
            sp.wait_ge(dma_x0, 16 * 10)
            for t in range(4, BT):
                if t >= XD:
                    # x buffers of u = t-XD free once all of u's groups ran
                    u = t - XD
                    sp.wait_ge(mm_done, g_ss(u))
                d = t % XD
                sp.dma_start(out=x_re_sb.ap()[:, d, :, :], in_=x_re_d.ap()[t]).then_inc(
                    dma_x, 16
                )
                sp.dma_start(out=x_im_sb.ap()[:, d, :, :], in_=x_im_d.ap()[t]).then_inc(
                    dma_x, 16
                )
                sp.dma_start(out=x_s_sb.ap()[:, d, :, :], in_=x_s_d.ap()[t]).then_inc(
                    dma_x, 16
                )

        @block.scalar
        def _(act):
            x_src = {"re": x_re_d, "im": x_im_d, "s": x_s_d}
            x_dst = {"re": x_re_sb, "im": x_im_sb, "s": x_s_sb}
            w_seq = (("re", wt_re_r, w_re_sb), ("im", wt_im_r, w_im_sb), ("s", wt_s_r, w_s_sb))

            def x0(kind, t):
                act.dma_start(
                    out=x_dst[kind].ap()[:, t % XD, :, :], in_=x_src[kind].ap()[t]
                ).then_inc(dma_x0, 16)

            def wchunk(src, dst, c):
                act.dma_start(
                    out=dst.ap()[:, c * WCH:(c + 1) * WCH, :],
                    in_=src[:, c * WCH:(c + 1) * WCH, :],
                ).then_inc(dma_w, 16)

            # exact arrival order on the serial DMA pool:
            # xr0, wre*4, xr1-3, wim*4, xi0-3, ws*4, xs0-3
            x0("re", 0)
            for c in range(NWC):
                wchunk(wt_re_r, w_re_sb, c)
            x0("re", 1); x0("re", 2); x0("re", 3)
            for c in range(NWC):
                wchunk(wt_im_r, w_im_sb, c)
            x0("im", 0); x0("im", 1); x0("im", 2); x0("im", 3)
            for c in range(NWC):
                wchunk(wt_s_r, w_s_sb, c)
            x0("s", 0); x0("s", 1); x0("s", 2); x0("s", 3)

            for t in range(BT):
                o = t % OD
                act.wait_ge(cmb_done, t + 1)
                act.dma_start(
                    out=out_d.ap()[0, t * 128:(t + 1) * 128, :],
                    in_=o_re_sb.ap()[:, o, :],
                ).then_inc(dma_out, 16)
                act.dma_start(
                    out=out_d.ap()[1, t * 128:(t + 1) * 128, :],
                    in_=o_im_sb.ap()[:, o, :],
                ).then_inc(dma_out, 16)

        @block.tensor
        def _(pe):
            # Ramp the PE clock on zeroed scratch while weights stream in.
            pe.wait_ge(warm_done, 1)
            for _ in range(N_WARM):
                pe.matmul(
                    out=p_warm.ap(),
                    lhsT=warm_sb.ap()[:, :128],
                    rhs=warm_sb.ap(),
                    start=True,
                    stop=True,
                )

            def group(xs, ws, ps, t, *, xwait, ppwait=None, wwait=None,
                      iiwait=None):
                if ppwait is not None:
                    pe.wait_ge(cmb_done, ppwait)
                if iiwait is not None:
                    pe.wait_ge(cmb_ii, iiwait)
                pe.wait_ge(*xwait)
                if wwait is not None:
                    pe.wait_ge(dma_w, 16 * wwait)
                for k in range(KT):
                    mm = pe.matmul(
                        out=ps.ap(),
                        lhsT=xs.ap()[:, t % XD, k, :],
                        rhs=ws.ap()[:, k, :],
                        start=(k == 0),
                        stop=(k == KT - 1),
                    )
                mm.then_inc(mm_done, 1)

            def xw(kind, t):
                if t <= 3:
                    return (dma_x0, 16 * X0_SLOT[(kind, t)])
                g = {"re": 1, "im": 2, "s": 3}[kind]
                return (dma_x, 16 * (3 * (t - 4) + g))

            # phase A: rr(0..3) — only w_re needed
            for t in range(4):
                group(x_re_sb, w_re_sb, p_rr_of(t), t,
                      xwait=xw("re", t), wwait=NWC if t == 0 else None)
            # phase B: ii(0..3) then ss(0..3) — w_s arrives last
            for t in range(4):
                group(x_im_sb, w_im_sb, p_ii_of(t), t,
                      xwait=xw("im", t), wwait=2 * NWC if t == 0 else None,
                      iiwait=t - 1 if t >= 2 else None)
            for t in range(4):
                group(x_s_sb, w_s_sb, p_ss_of(t), t,
                      xwait=xw("s", t), wwait=3 * NWC if t == 0 else None,
                      ppwait=t - 1 if t >= 2 else None)
            # phase C: steady state
            for t in range(4, BT):
                group(x_re_sb, w_re_sb, p_rr_of(t), t,
                      xwait=xw("re", t), ppwait=t - 3)
                group(x_im_sb, w_im_sb, p_ii_of(t), t,
                      xwait=xw("im", t), iiwait=t - 1)
                group(x_s_sb, w_s_sb, p_ss_of(t), t,
                      xwait=xw("s", t), ppwait=t - 1)

        @block.vector
        def _(dve):
            dve.memset(warm_sb.ap(), 0).then_inc(warm_done, 1)

            def rr_step(t):
                dve.wait_ge(mm_done, g_rr(t))
                dve.tensor_copy(r_sb.ap()[:, t % 4, :], p_rr_of(t).ap())

            def ii_step(t):
                if t >= OD:
                    # staging buffer o reused from t-OD: its stores flushed
                    dve.wait_ge(dma_out, 32 * (t - OD + 1))
                dve.wait_ge(mm_done, g_ii(t))
                dve.tensor_sub(
                    o_re_sb.ap()[:, t % OD, :], r_sb.ap()[:, t % 4, :],
                    p_ii_of(t).ap()
                )
                # s2 = rr + ii frees the ii bank before ss(t+2) reuses it
                dve.tensor_add(
                    s2_sb.ap()[:, t % 4, :], r_sb.ap()[:, t % 4, :],
                    p_ii_of(t).ap()
                ).then_inc(cmb_ii, 1)

            def ss_step(t):
                dve.wait_ge(mm_done, g_ss(t))
                dve.tensor_sub(
                    o_im_sb.ap()[:, t % OD, :], p_ss_of(t).ap(),
                    s2_sb.ap()[:, t % 4, :]
                ).then_inc(cmb_done, 1)

            # phase B mirror: PE runs rr(0..3), ii(0..3), ss(0..3)
            for t in range(4):
                rr_step(t)
            for t in range(4):
                ii_step(t)
            for t in range(4):
                ss_step(t)
            for t in range(4, BT):
                rr_step(t)
                ii_step(t)
                ss_step(t)

    return nc


_NC = None
LAST_RES = None  # last BassKernelResults (exec_time_ns when BASS_TRACE=1)


def _get_nc() -> bass.Bass:
    global _NC
    if _NC is None:
        _NC = build_nc()
    return _NC


def _arrange_x(x16_half):
    """[4096, 2048] bf16 -> [BT, 128, KT, 128] tile-major contiguous."""
    # element (t*128+c, k*128+p) -> [t, p, k, c]
    a = x16_half.reshape(BT, 128, KT, 128)  # [t, c, k, p]
    return np.ascontiguousarray(a.transpose(0, 3, 2, 1))


def kernel(x_re, x_im, w_re, w_im):
    x_re = np.asarray(x_re, dtype=np.float32)
    x_im = np.asarray(x_im, dtype=np.float32)
    w_re = np.asarray(w_re, dtype=np.float32)
    w_im = np.asarray(w_im, dtype=np.float32)

    x_re16 = x_re.astype(BF16_NP)
    x_im16 = x_im.astype(BF16_NP)
    x_s16 = (x_re + x_im).astype(BF16_NP)

    x_re_h = [_arrange_x(x_re16[h * B_SH:(h + 1) * B_SH]) for h in range(B_SHARDS)]
    x_im_h = [_arrange_x(x_im16[h * B_SH:(h + 1) * B_SH]) for h in range(B_SHARDS)]
    x_s_h = [_arrange_x(x_s16[h * B_SH:(h + 1) * B_SH]) for h in range(B_SHARDS)]

    wt_re16 = w_re.astype(BF16_NP).T  # [in, out]
    wt_im16 = w_im.astype(BF16_NP).T
    wt_s16 = (w_re + w_im).astype(BF16_NP).T
    wt_re_q = [np.ascontiguousarray(wt_re16[:, q * O_SH:(q + 1) * O_SH]) for q in range(O_SHARDS)]
    wt_im_q = [np.ascontiguousarray(wt_im16[:, q * O_SH:(q + 1) * O_SH]) for q in range(O_SHARDS)]
    wt_s_q = [np.ascontiguousarray(wt_s16[:, q * O_SH:(q + 1) * O_SH]) for q in range(O_SHARDS)]

    in_maps = []
    for c in range(N_CORES):
        bs, os_ = c // O_SHARDS, c % O_SHARDS
        in_maps.append(
            {
                "x_re_d": x_re_h[bs],
                "x_im_d": x_im_h[bs],
                "x_s_d": x_s_h[bs],
                "wt_re": wt_re_q[os_],
                "wt_im": wt_im_q[os_],
                "wt_s": wt_s_q[os_],
            }
        )

    nc = _get_nc()
    res = run_bass_kernel_spmd(nc, in_maps, core_ids=list(range(N_CORES)))
    global LAST_RES
    LAST_RES = res

    out = np.empty((2, BATCH, OUT_F), dtype=np.float32)
    for c in range(N_CORES):
        bs, os_ = c // O_SHARDS, c % O_SHARDS
        out[:, bs * B_SH:(bs + 1) * B_SH, os_ * O_SH:(os_ + 1) * O_SH] = (
            res.results[c]["out"]
        )
    return out
